# revision 4
# baseline (speedup 1.0000x reference)
"""GAT layer kernel for Trainium2, data-parallel over batch across 8 NeuronCores.

Per core (one batch b):
  Wh   = h_b @ W                     (PE, via PE-transposed h)
  s_i  = Wh @ a[:F],  s_j = Wh @ a[F:]
  E    = exp(leaky_relu(s_i[:,None] + s_j[None,:]))
       = exp(si + max(alpha*sj + (alpha-1)*si, sj))   -- one DVE op + one ACT op per tile
  Z    = rowsum(E)                   (free via ACT accum_out)
  c    = sum_i E[i,:] / Z[i]         (PE matmul with lhsT = 1/Z)
  out  = (1/N) * sum_j c[j] * Wh[j,:]
"""
import sys
sys.path.insert(0, "/opt/trn_rl_repo")
from contextlib import ExitStack

import numpy as np

import concourse.bass as bass
import concourse.tile as tile
from concourse import bacc, mybir
from concourse.bass_utils import run_bass_kernel_spmd
from concourse.masks import make_identity

N, K, F, P, T = 2048, 128, 64, 128, 16  # nodes, f_in, f_out, partitions, row tiles
NC_ = 4            # 512-wide column chunks per N
ALPHA = 0.2
NCORES = 8
FP = mybir.dt.float32
AF = mybir.ActivationFunctionType
OP = mybir.AluOpType
ts = bass.ts


def emit_batch(tc, outd, hb, Wd, ad, consts, ident, W_sb, a1, a2):
    """Emit the full per-batch computation (one repetition)."""
    nc = tc.nc
    with ExitStack() as ctx:
        big = ctx.enter_context(tc.tile_pool(name="big", bufs=1))
        upool = ctx.enter_context(tc.tile_pool(name="u", bufs=2))
        epool = ctx.enter_context(tc.tile_pool(name="e", bufs=2))
        zpool = ctx.enter_context(tc.tile_pool(name="z", bufs=4))
        small = ctx.enter_context(tc.tile_pool(name="small", bufs=2))
        psum = ctx.enter_context(
            tc.tile_pool(name="ps", bufs=4, space=bass.MemorySpace.PSUM)
        )
        cpsum = ctx.enter_context(
            tc.tile_pool(name="cps", bufs=1, space=bass.MemorySpace.PSUM)
        )

        # h load: hbuf[p, t*K + k] = hb[t*P + p, k]
        hbuf = big.tile([P, N], FP)
        nc.sync.dma_start(
            hbuf[:].rearrange("p (t k) -> p t k", t=T),
            hb.rearrange("(t p) k -> p t k", p=P),
        )

        # hT[k, n] via 16 PE transposes
        hT = big.tile([P, N], FP)
        for t in range(T):
            ps = psum.tile([P, P], FP, tag="ps", name="ps_tr")
            nc.tensor.transpose(ps[:], hbuf[:, ts(t, P)], ident[:])
            if t % 2 == 0:
                nc.vector.tensor_copy(hT[:, ts(t, P)], ps[:])
            else:
                nc.scalar.copy(hT[:, ts(t, P)], ps[:])

        # WhT[f, n] = W.T @ hT
        WhT = big.tile([F, N], FP)
        for c in range(NC_):
            ps = psum.tile([F, 512], FP, tag="ps", name="ps_whT")
            nc.tensor.matmul(ps[:], W_sb[:], hT[:, ts(c, 512)], start=True, stop=True)
            nc.vector.tensor_copy(WhT[:, ts(c, 512)], ps[:])

        # SJ[p, j] = s_j[j] (bcast over partitions) via lhsT = a2 replicated
        a2_rep = small.tile([F, P], FP, tag="a2rep")
        nc.scalar.activation(a2_rep[:], ident[0:F, :], AF.Identity, bias=a2[:, 0:1], scale=0.0)
        SJ = big.tile([P, N], FP)
        SJa = big.tile([P, N], FP)
        for c in range(NC_):
            ps = psum.tile([P, 512], FP, tag="ps", name="ps_sj")
            nc.tensor.matmul(ps[:], a2_rep[:], WhT[:, ts(c, 512)], start=True, stop=True)
            nc.scalar.copy(SJ[:, ts(c, 512)], ps[:])
            nc.vector.tensor_scalar_mul(SJa[:, ts(c, 512)], ps[:], ALPHA)

        # si in column layout [P, T]: si[p, t] = s_i[t*P + p]
        si_ps = psum.tile([P, T], FP, tag="ps", name="ps_si")
        for t in range(T):
            nc.tensor.matmul(si_ps[:, t : t + 1], WhT[:, ts(t, P)], a1[:], start=True, stop=True)
        si = small.tile([P, T], FP, tag="si_sb")
        nc.vector.tensor_copy(si[:], si_ps[:])
        si_m = small.tile([P, T], FP, tag="si_m")
        nc.scalar.mul(si_m[:], si[:], ALPHA - 1.0)

        # Wh[n, f] natural layout (for epilogue), off critical path
        Wh = big.tile([P, T * F], FP)
        for t in range(T):
            ps = psum.tile([P, F], FP, tag="ps", name="ps_wh")
            nc.tensor.matmul(ps[:], hT[:, ts(t, P)], W_sb[:], start=True, stop=True)
            if t % 2 == 0:
                nc.vector.tensor_copy(Wh[:, ts(t, F)], ps[:])
            else:
                nc.scalar.copy(Wh[:, ts(t, F)], ps[:])

        # ---- main N^2 loop over 16 row tiles ----
        c_ps = [
            cpsum.tile([1, 512], FP, tag=f"c{c}", name=f"c_ps{c}") for c in range(NC_)
        ]
        for t in range(T):
            u = upool.tile([P, N], FP, tag="u")
            # u = max(alpha*sj + (alpha-1)*si_t, sj)
            nc.vector.scalar_tensor_tensor(
                u[:], SJa[:], si_m[:, t : t + 1], SJ[:], OP.add, OP.max
            )
            E = epool.tile([P, N], FP, tag="E")
            Z = zpool.tile([P, 1], FP, tag="Z")
            # E = exp(u + si_t), Z = rowsum(E)
            nc.scalar.activation(E[:], u[:], AF.Exp, bias=si[:, t : t + 1], scale=1.0, accum_out=Z[:])
            invZ = zpool.tile([P, 1], FP, tag="invZ")
            nc.vector.reciprocal(invZ[:], Z[:])
            for c in range(NC_):
                nc.tensor.matmul(
                    c_ps[c][:], invZ[:], E[:, ts(c, 512)],
                    start=(t == 0), stop=(t == T - 1),
                )

        # ---- epilogue: c (row layout) -> column layout -> weighted sum of Wh ----
        c_row = small.tile([1, N], FP, tag="crow")
        for c in range(NC_):
            if c % 2 == 0:
                nc.scalar.copy(c_row[:, ts(c, 512)], c_ps[c][:])
            else:
                nc.vector.tensor_copy(c_row[:, ts(c, 512)], c_ps[c][:])
        ccol_ps = psum.tile([P, T], FP, tag="ps", name="ps_ccol")
        for t in range(T):
            nc.tensor.transpose(ccol_ps[:, t : t + 1], c_row[:, ts(t, P)], ident[0:1, 0:1])
        c_col = small.tile([P, T], FP, tag="ccol_sb")
        nc.vector.tensor_copy(c_col[:], ccol_ps[:])

        g_ps = psum.tile([F, 1], FP, tag="ps", name="ps_g")
        for t in range(T):
            nc.tensor.matmul(
                g_ps[:], Wh[:, ts(t, F)], c_col[:, t : t + 1],
                start=(t == 0), stop=(t == T - 1),
            )
        out_sb = small.tile([F, 1], FP, tag="out")
        nc.scalar.mul(out_sb[:], g_ps[:], 1.0 / N)
        nc.sync.dma_start(outd[:], out_sb[:])


def build(reps: int = 1):
    nc = bacc.Bacc(
        "TRN2", target_bir_lowering=False, debug=False,
        enable_asserts=True, num_devices=NCORES,
    )
    hb = nc.dram_tensor("hb", [N, K], FP, kind="ExternalInput").ap()
    Wd = nc.dram_tensor("W", [K, F], FP, kind="ExternalInput").ap()
    ad = nc.dram_tensor("a", [2 * F, 1], FP, kind="ExternalInput").ap()
    outd = nc.dram_tensor("out", [F, 1], FP, kind="ExternalOutput").ap()

    with tile.TileContext(nc) as tc:
        with ExitStack() as ctx:
            consts = ctx.enter_context(tc.tile_pool(name="consts", bufs=1))
            ident = consts.tile([P, P], FP)
            make_identity(nc, ident[:])
            # pull the exp ACT table load ahead of the critical path
            warm = consts.tile([P, 1], FP)
            nc.scalar.activation(warm[:], ident[:, 0:1], AF.Exp)
            W_sb = consts.tile([K, F], FP)
            nc.sync.dma_start(W_sb[:], Wd[:])
            a1 = consts.tile([F, 1], FP)
            nc.sync.dma_start(a1[:], ad[0:F, :])
            a2 = consts.tile([F, 1], FP)
            nc.sync.dma_start(a2[:], ad[F : 2 * F, :])
            for _ in range(reps):
                emit_batch(tc, outd, hb, Wd, ad, consts, ident, W_sb, a1, a2)
    nc.compile()
    return nc


_nc_cache = {}


def _get_nc(reps: int = 1):
    if reps not in _nc_cache:
        _nc_cache[reps] = build(reps)
    return _nc_cache[reps]


def kernel(h: np.ndarray, W: np.ndarray, a: np.ndarray) -> np.ndarray:
    assert h.shape == (NCORES, N, K) and W.shape == (K, F) and a.shape == (2 * F,)
    nc = _get_nc(1)
    in_maps = [
        {
            "hb": np.ascontiguousarray(h[b], dtype=np.float32),
            "W": np.ascontiguousarray(W, dtype=np.float32),
            "a": np.ascontiguousarray(a.reshape(2 * F, 1), dtype=np.float32),
        }
        for b in range(NCORES)
    ]
    res = run_bass_kernel_spmd(nc, in_maps, core_ids=list(range(NCORES)))
    out = np.stack([res.results[b]["out"].reshape(F) for b in range(NCORES)])
    return out.astype(np.float32)


# revision 10
# speedup vs baseline: 1.3298x; 1.3298x over previous
"""GAT layer kernel for Trainium2, data-parallel over batch across 8 NeuronCores.

Per core (one batch b):
  Wh   = h_b @ W                     (PE, via PE-transposed h)
  s_i  = Wh @ a[:F],  s_j = Wh @ a[F:]
  E    = exp(leaky_relu(s_i[:,None] + s_j[None,:]))
       = exp(si + max(alpha*sj + (alpha-1)*si, sj))   -- one DVE op + one ACT op per tile
  Z    = rowsum(E)                   (free via ACT accum_out)
  c    = sum_i E[i,:] / Z[i]         (PE matmul with lhsT = 1/Z)
  out  = (1/N) * sum_j c[j] * Wh[j,:]
"""
import sys
sys.path.insert(0, "/opt/trn_rl_repo")
from contextlib import ExitStack

import numpy as np

import concourse.bass as bass
import concourse.tile as tile
from concourse import bacc, mybir
from concourse.bass_utils import run_bass_kernel_spmd
from concourse.masks import make_identity

N, K, F, P, T = 2048, 128, 64, 128, 16  # nodes, f_in, f_out, partitions, row tiles
NC_ = 4            # 512-wide column chunks per N
ALPHA = 0.2
NCORES = 8
FP = mybir.dt.float32
AF = mybir.ActivationFunctionType
OP = mybir.AluOpType
ts = bass.ts
GP_TILES = set()  # TensorScalarPtr is not a valid Pool-engine op on v3
COLSUM_MODE = "f32r"  # one of "f32", "bf16", "f32r"
BF = mybir.dt.bfloat16
FR = mybir.dt.float32r


def mm_cast(ap):
    """Bitcast an fp32 AP to float32r for full-rate PE streaming."""
    return ap.bitcast(FR)


def emit_batch(tc, outd, hb, Wd, ad, consts, ident, W_sb, a1, a2):
    """Emit the full per-batch computation (one repetition)."""
    nc = tc.nc
    with ExitStack() as ctx:
        big = ctx.enter_context(tc.tile_pool(name="big", bufs=1))
        upool = ctx.enter_context(tc.tile_pool(name="u", bufs=4))
        epool = ctx.enter_context(tc.tile_pool(name="e", bufs=3))
        zpool = ctx.enter_context(tc.tile_pool(name="z", bufs=4))
        small = ctx.enter_context(tc.tile_pool(name="small", bufs=2))
        psum = ctx.enter_context(
            tc.tile_pool(name="ps", bufs=4, space=bass.MemorySpace.PSUM)
        )
        cpsum = ctx.enter_context(
            tc.tile_pool(name="cps", bufs=1, space=bass.MemorySpace.PSUM)
        )

        # h load: hbuf[p, t*K + k] = hb[t*P + p, k]
        hbuf = big.tile([P, N], FP)
        hb3 = hb.rearrange("(t p) k -> p t k", p=P)
        hbuf3 = hbuf[:].rearrange("p (t k) -> p t k", t=T)
        for g in range(4):
            nc.sync.dma_start(hbuf3[:, 4 * g : 4 * g + 4, :], hb3[:, 4 * g : 4 * g + 4, :])

        # PE warmup: ~10 dummy matmuls on ident during the h DMA keep the
        # clock ramp going so prologue matmuls run at full rate
        warm_ps = psum.tile([P, 512], FP, tag="ps", name="ps_warm")
        for _ in range(10):
            nc.tensor.matmul(warm_ps[:, 0:P], ident[:], ident[:], start=True, stop=True)

        # hT[k, n] via 16 PE transposes, 4 per PSUM bank, one copy per group
        hT = big.tile([P, N], FP)
        for g in range(4):
            ps = psum.tile([P, 512], FP, tag="ps", name="ps_tr")
            for q in range(4):
                t = 4 * g + q
                nc.tensor.transpose(ps[:, ts(q, P)], hbuf[:, ts(t, P)], ident[:])
            if g % 2 == 0:
                nc.vector.tensor_copy(hT[:, ts(g, 512)], ps[:])
            else:
                nc.scalar.copy(hT[:, ts(g, 512)], ps[:])

        # WhT[f, n] = W.T @ hT ; SJ = bcast(s_j) ; si column layout.
        # Interleaved per 512-chunk so the first stt can start as early
        # as possible.
        a2_rep = small.tile([F, P], FP, tag="a2rep")
        nc.scalar.activation(a2_rep[:], ident[0:F, :], AF.Identity, bias=a2[:, 0:1], scale=0.0)
        WhT = big.tile([F, N], FP)
        SJ = big.tile([P, N], FP)
        SJa = big.tile([P, N], FP)
        si_ps = psum.tile([P, T], FP, tag="ps", name="ps_si")
        for c in range(NC_):
            ps = psum.tile([F, 512], FP, tag="ps", name="ps_whT")
            nc.tensor.matmul(ps[:], W_sb[:], hT[:, ts(c, 512)], start=True, stop=True)
            if c % 2 == 0:
                nc.scalar.copy(WhT[:, ts(c, 512)], ps[:])
            else:
                nc.vector.tensor_copy(WhT[:, ts(c, 512)], ps[:])
            ps2 = psum.tile([P, 512], FP, tag="ps", name="ps_sj")
            nc.tensor.matmul(ps2[:], a2_rep[:], WhT[:, ts(c, 512)], start=True, stop=True)
            if c % 2 == 0:
                nc.scalar.copy(SJ[:, ts(c, 512)], ps2[:])
            else:
                nc.vector.tensor_copy(SJ[:, ts(c, 512)], ps2[:])
            nc.vector.tensor_scalar_mul(SJa[:, ts(c, 512)], ps2[:], ALPHA)
            for t in range(4 * c, 4 * c + 4):
                nc.tensor.matmul(si_ps[:, t : t + 1], WhT[:, ts(t, P)], a1[:], start=True, stop=True)
        si = small.tile([P, T], FP, tag="si_sb")
        nc.vector.tensor_copy(si[:], si_ps[:])
        si_m = small.tile([P, T], FP, tag="si_m")
        nc.scalar.mul(si_m[:], si[:], ALPHA - 1.0)

        # Wh[n, f] natural layout (for epilogue), off critical path
        Wh = big.tile([P, T * F], FP)
        for g in range(2):
            ps = psum.tile([P, 512], FP, tag="ps", name="ps_wh")
            for q in range(8):
                t = 8 * g + q
                nc.tensor.matmul(ps[:, ts(q, F)], hT[:, ts(t, P)], W_sb[:], start=True, stop=True)
            if g == 0:
                nc.vector.tensor_copy(Wh[:, ts(g, 512)], ps[:])
            else:
                nc.scalar.copy(Wh[:, ts(g, 512)], ps[:])

        # ---- main N^2 loop over 16 row tiles ----
        c_ps = [
            cpsum.tile([1, 512], FP, tag=f"c{c}", name=f"c_ps{c}") for c in range(NC_)
        ]
        for t in range(T):
            u = upool.tile([P, N], FP, tag="u")
            # u = max(alpha*sj + (alpha-1)*si_t, sj)
            eng = nc.gpsimd if (t % 16) in GP_TILES else nc.vector
            eng.scalar_tensor_tensor(
                u[:], SJa[:], si_m[:, t : t + 1], SJ[:], OP.add, OP.max
            )
            if COLSUM_MODE == "bf16":
                e_dt = BF
            elif COLSUM_MODE == "f32r":
                e_dt = FR
            else:
                e_dt = FP
            E = epool.tile([P, N], e_dt, tag="E")
            Z = zpool.tile([P, 1], FP, tag="Z")
            # E = exp(u + si_t), Z = rowsum(E)
            nc.scalar.activation(E[:], u[:], AF.Exp, bias=si[:, t : t + 1], scale=1.0, accum_out=Z[:])
            invZ = zpool.tile([P, 1], FP, tag="invZ")
            nc.vector.reciprocal(invZ[:], Z[:])
            if COLSUM_MODE != "f32":
                invZc = zpool.tile([P, 1], e_dt, tag="invZc")
                nc.vector.tensor_copy(invZc[:], invZ[:])
            else:
                invZc = invZ
            for c in range(NC_):
                nc.tensor.matmul(
                    c_ps[c][:], invZc[:], E[:, ts(c, 512)],
                    start=(t == 0), stop=(t == T - 1),
                )

        # ---- epilogue: c (row layout) -> column layout -> weighted sum of Wh ----
        c_row = small.tile([1, N], FP, tag="crow")
        for c in range(NC_):
            if c % 2 == 0:
                nc.scalar.copy(c_row[:, ts(c, 512)], c_ps[c][:])
            else:
                nc.vector.tensor_copy(c_row[:, ts(c, 512)], c_ps[c][:])
        ccol_ps = psum.tile([P, T], FP, tag="ps", name="ps_ccol")
        for t in range(T):
            nc.tensor.transpose(ccol_ps[:, t : t + 1], c_row[:, ts(t, P)], ident[0:1, 0:1])
        c_col = small.tile([P, T], FP, tag="ccol_sb")
        nc.vector.tensor_copy(c_col[:], ccol_ps[:])

        g_ps = psum.tile([F, 1], FP, tag="ps", name="ps_g")
        for t in range(T):
            nc.tensor.matmul(
                g_ps[:], Wh[:, ts(t, F)], c_col[:, t : t + 1],
                start=(t == 0), stop=(t == T - 1),
            )
        out_sb = small.tile([F, 1], FP, tag="out")
        nc.scalar.mul(out_sb[:], g_ps[:], 1.0 / N)
        nc.sync.dma_start(outd[:], out_sb[:])


def build(reps: int = 1):
    nc = bacc.Bacc(
        "TRN2", target_bir_lowering=False, debug=False,
        enable_asserts=True, num_devices=NCORES,
    )
    hb = nc.dram_tensor("hb", [N, K], FP, kind="ExternalInput").ap()
    Wd = nc.dram_tensor("W", [K, F], FP, kind="ExternalInput").ap()
    ad = nc.dram_tensor("a", [2 * F, 1], FP, kind="ExternalInput").ap()
    outd = nc.dram_tensor("out", [F, 1], FP, kind="ExternalOutput").ap()

    with tile.TileContext(nc) as tc:
        with ExitStack() as ctx:
            consts = ctx.enter_context(tc.tile_pool(name="consts", bufs=1))
            ident = consts.tile([P, P], FP)
            make_identity(nc, ident[:])
            # pull the exp ACT table load ahead of the critical path
            warm = consts.tile([P, 1], FP)
            nc.scalar.activation(warm[:], ident[:, 0:1], AF.Exp)
            W_sb = consts.tile([K, F], FP)
            nc.sync.dma_start(W_sb[:], Wd[:])
            a1 = consts.tile([F, 1], FP)
            nc.sync.dma_start(a1[:], ad[0:F, :])
            a2 = consts.tile([F, 1], FP)
            nc.sync.dma_start(a2[:], ad[F : 2 * F, :])
            for _ in range(reps):
                emit_batch(tc, outd, hb, Wd, ad, consts, ident, W_sb, a1, a2)
    nc.compile()
    return nc


_nc_cache = {}


def _get_nc(reps: int = 1):
    if reps not in _nc_cache:
        _nc_cache[reps] = build(reps)
    return _nc_cache[reps]


def kernel(h: np.ndarray, W: np.ndarray, a: np.ndarray) -> np.ndarray:
    assert h.shape == (NCORES, N, K) and W.shape == (K, F) and a.shape == (2 * F,)
    nc = _get_nc(1)
    in_maps = [
        {
            "hb": np.ascontiguousarray(h[b], dtype=np.float32),
            "W": np.ascontiguousarray(W, dtype=np.float32),
            "a": np.ascontiguousarray(a.reshape(2 * F, 1), dtype=np.float32),
        }
        for b in range(NCORES)
    ]
    res = run_bass_kernel_spmd(nc, in_maps, core_ids=list(range(NCORES)))
    out = np.stack([res.results[b]["out"].reshape(F) for b in range(NCORES)])
    return out.astype(np.float32)


# revision 24
# speedup vs baseline: 278.6200x; 209.5126x over previous
"""GAT layer kernel for Trainium2, data-parallel over batch across 8 NeuronCores.

Per core (one batch b):
  Wh   = h_b @ W                     (PE, via PE-transposed h)
  s_i  = Wh @ a[:F],  s_j = Wh @ a[F:]
  E    = exp(leaky_relu(s_i[:,None] + s_j[None,:]))
       = exp(si + max(alpha*sj + (alpha-1)*si, sj))   -- one DVE op + one ACT op per tile
  Z    = rowsum(E)                   (free via ACT accum_out)
  c    = sum_i E[i,:] / Z[i]         (PE matmul with lhsT = 1/Z)
  out  = (1/N) * sum_j c[j] * Wh[j,:]
"""
import sys
sys.path.insert(0, "/opt/trn_rl_repo")
from contextlib import ExitStack

import numpy as np

import concourse.bass as bass
import concourse.tile as tile
from concourse import bacc, mybir
from concourse.bass_utils import run_bass_kernel_spmd
from concourse.masks import make_identity

N, K, F, P, T = 2048, 128, 64, 128, 16  # nodes, f_in, f_out, partitions, row tiles
NC_ = 4            # 512-wide column chunks per N
ALPHA = 0.2
NCORES = 8
FP = mybir.dt.float32
AF = mybir.ActivationFunctionType
OP = mybir.AluOpType
ts = bass.ts
GP_TILES = set()  # TensorScalarPtr is not a valid Pool-engine op on v3
COLSUM_MODE = "f32r"  # one of "f32", "bf16", "f32r"
ALL_F32R = False  # f32r everywhere was +30x error (1.3e-4); colsum-only f32r is ~4e-6
BF = mybir.dt.bfloat16
FR = mybir.dt.float32r


def mm_cast(ap):
    """Bitcast an fp32 AP to float32r for full-rate PE streaming."""
    return ap.bitcast(FR)


def emit_batch(tc, outd, hb, Wd, ad, consts, ident, W_sb, a1, a2):
    """Emit the full per-batch computation (one repetition)."""
    nc = tc.nc
    with ExitStack() as ctx:
        big = ctx.enter_context(tc.tile_pool(name="big", bufs=1))
        upool = ctx.enter_context(tc.tile_pool(name="u", bufs=6))
        epool = ctx.enter_context(tc.tile_pool(name="e", bufs=4))
        zpool = ctx.enter_context(tc.tile_pool(name="z", bufs=6))
        small = ctx.enter_context(tc.tile_pool(name="small", bufs=2))
        pro_psum_ctx = tc.tile_pool(name="ps", bufs=7, space=bass.MemorySpace.PSUM)
        psum = pro_psum_ctx.__enter__()

        # h load: hbuf[p, t*K + k] = hb[t*P + p, k]
        hbuf = big.tile([P, N], FP)
        hb3 = hb.rearrange("(t p) k -> p t k", p=P)
        hbuf3 = hbuf[:].rearrange("p (t k) -> p t k", t=T)
        for g in range(4):
            dma_eng = nc.sync if g % 2 == 0 else nc.scalar
            dma_eng.dma_start(hbuf3[:, 4 * g : 4 * g + 4, :], hb3[:, 4 * g : 4 * g + 4, :])

        # PE warmup: ~10 dummy matmuls on ident during the h DMA keep the
        # clock ramp going so prologue matmuls run at full rate
        warm_ps = psum.tile([P, 512], FP, tag="tr", bufs=2, name="ps_warm")
        for _ in range(10):
            nc.tensor.matmul(warm_ps[:, 0:P], ident[:], ident[:], start=True, stop=True)

        MMD = FR if ALL_F32R else FP  # dtype for matmul-operand SBUF tiles
        # hT[k, n] via 16 PE transposes, 4 per PSUM bank, one copy per group
        hT = big.tile([P, N], MMD)
        for g in range(4):
            ps = psum.tile([P, 512], FP, tag="tr", bufs=2, name="ps_tr")
            for q in range(4):
                t = 4 * g + q
                nc.tensor.transpose(ps[:, ts(q, P)], hbuf[:, ts(t, P)], ident[:])
            if g % 2 == 0:
                nc.vector.tensor_copy(hT[:, ts(g, 512)], ps[:])
            else:
                nc.scalar.copy(hT[:, ts(g, 512)], ps[:])

        # WhT[f, n] = W.T @ hT ; SJ = bcast(s_j) ; si column layout.
        # Interleaved per 512-chunk so the first stt can start as early
        # as possible.
        a2_rep = small.tile([F, P], MMD, tag="a2rep")
        nc.scalar.activation(a2_rep[:], ident[0:F, :], AF.Identity, bias=a2[:, 0:1], scale=0.0)
        if ALL_F32R:
            W_mm = small.tile([K, F], MMD, tag="W_mm")
            nc.vector.tensor_copy(W_mm[:], W_sb[:])
            a1_mm = small.tile([F, 2], MMD, tag="a1_mm")
            nc.vector.tensor_copy(a1_mm[:, 0:1], a1[:])
            nc.vector.tensor_copy(a1_mm[:, 1:2], a1[:])
        else:
            W_mm, a1_mm = W_sb, a1
        WhT = big.tile([F, N], MMD)
        SJ = big.tile([P, N], FP)
        SJa = big.tile([P, N], FP)
        si = small.tile([P, T], FP, tag="si_sb")
        si_m = small.tile([P, T], FP, tag="si_m")
        si_ps = psum.tile([P, 2 * T if ALL_F32R else T], FP, tag="si", bufs=1, name="ps_si")
        for c in range(NC_):
            ps = psum.tile([F, 512], FP, tag="whT", bufs=2, name="ps_whT")
            nc.tensor.matmul(ps[:], W_mm[:], hT[:, ts(c, 512)], start=True, stop=True)
            if c % 2 == 0:
                nc.scalar.copy(WhT[:, ts(c, 512)], ps[:])
            else:
                nc.vector.tensor_copy(WhT[:, ts(c, 512)], ps[:])
            ps2 = psum.tile([P, 512], FP, tag="sj", bufs=2, name="ps_sj")
            nc.tensor.matmul(ps2[:], a2_rep[:], WhT[:, ts(c, 512)], start=True, stop=True)
            if c % 2 == 0:
                nc.scalar.copy(SJ[:, ts(c, 512)], ps2[:])
                nc.vector.tensor_scalar_mul(SJa[:, ts(c, 512)], ps2[:], ALPHA)
            else:
                nc.vector.tensor_copy(SJ[:, ts(c, 512)], ps2[:])
                nc.scalar.mul(SJa[:, ts(c, 512)], ps2[:], ALPHA)
            for t in range(4 * c, 4 * c + 4):
                if ALL_F32R:
                    nc.tensor.matmul(si_ps[:, 2 * t : 2 * t + 2], WhT[:, ts(t, P)], a1_mm[:], start=True, stop=True)
                else:
                    nc.tensor.matmul(si_ps[:, t : t + 1], WhT[:, ts(t, P)], a1_mm[:], start=True, stop=True)
            if not ALL_F32R:
                nc.vector.tensor_copy(si[:, 4 * c : 4 * c + 4], si_ps[:, 4 * c : 4 * c + 4])
                nc.scalar.mul(si_m[:, 4 * c : 4 * c + 4], si[:, 4 * c : 4 * c + 4], ALPHA - 1.0)
        if ALL_F32R:
            nc.vector.tensor_copy(
                si[:].rearrange("p (t one) -> p t one", one=1),
                si_ps[:].rearrange("p (t two) -> p t two", two=2)[:, :, 0:1],
            )
            nc.scalar.mul(si_m[:], si[:], ALPHA - 1.0)

        # Wh[n, f] natural layout — emitted here but only needed by the epilogue
        Wh = big.tile([P, T * F], FP)
        for g in range(2):
            ps = psum.tile([P, 512], FP, tag="tr", bufs=2, name="ps_wh")
            for q in range(8):
                t = 8 * g + q
                nc.tensor.matmul(ps[:, ts(q, F)], hT[:, ts(t, P)], W_mm[:], start=True, stop=True)
            if g == 0:
                nc.vector.tensor_copy(Wh[:, ts(g, 512)], ps[:])
            else:
                nc.scalar.copy(Wh[:, ts(g, 512)], ps[:])

        # ---- main N^2 loop over 16 row tiles ----
        pro_psum_ctx.__exit__(None, None, None)
        cpsum = ctx.enter_context(
            tc.tile_pool(name="cps", bufs=1, space=bass.MemorySpace.PSUM)
        )
        epi_psum = ctx.enter_context(
            tc.tile_pool(name="eps", bufs=2, space=bass.MemorySpace.PSUM)
        )
        c_ps = [
            cpsum.tile([1, 512], FP, tag=f"c{c}", name=f"c_ps{c}") for c in range(NC_)
        ]
        for t in range(T):
            u = upool.tile([P, N], FP, tag="u")
            # u = max(alpha*sj + (alpha-1)*si_t, sj)
            if t < 3:
                # chunked: start as soon as each SJ/SJa chunk is ready
                for c in range(NC_):
                    nc.vector.scalar_tensor_tensor(
                        u[:, ts(c, 512)], SJa[:, ts(c, 512)], si_m[:, t : t + 1],
                        SJ[:, ts(c, 512)], OP.add, OP.max,
                    )
            else:
                nc.vector.scalar_tensor_tensor(
                    u[:], SJa[:], si_m[:, t : t + 1], SJ[:], OP.add, OP.max
                )
            if COLSUM_MODE == "bf16":
                e_dt = BF
            elif COLSUM_MODE == "f32r":
                e_dt = FR
            else:
                e_dt = FP
            E = epool.tile([P, N], e_dt, tag="E")
            Z = zpool.tile([P, 1], FP, tag="Z")
            # E = exp(u + si_t), Z = rowsum(E)
            nc.scalar.activation(E[:], u[:], AF.Exp, bias=si[:, t : t + 1], scale=1.0, accum_out=Z[:])
            invZ = zpool.tile([P, 1], FP, tag="invZ")
            nc.vector.reciprocal(invZ[:], Z[:])
            if COLSUM_MODE != "f32":
                invZc = zpool.tile([P, 1], e_dt, tag="invZc")
                nc.vector.tensor_copy(invZc[:], invZ[:])
            else:
                invZc = invZ
            for c in range(NC_):
                nc.tensor.matmul(
                    c_ps[c][:], invZc[:], E[:, ts(c, 512)],
                    start=(t == 0), stop=(t == T - 1),
                )

        # ---- epilogue: c (row layout) -> column layout -> weighted sum of Wh ----
        c_row = small.tile([1, N], FP, tag="crow")
        for c in range(NC_):
            if c % 2 == 0:
                nc.scalar.copy(c_row[:, ts(c, 512)], c_ps[c][:])
            else:
                nc.vector.tensor_copy(c_row[:, ts(c, 512)], c_ps[c][:])
        ccol_ps = epi_psum.tile([P, T], FP, tag="eps", name="ps_ccol")
        for t in range(T):
            nc.tensor.transpose(ccol_ps[:, t : t + 1], c_row[:, ts(t, P)], ident[0:1, 0:1])
        c_col = small.tile([P, T], FP, tag="ccol_sb")
        nc.vector.tensor_copy(c_col[:], ccol_ps[:])

        g_ps = epi_psum.tile([F, 1], FP, tag="eps", name="ps_g")
        for t in range(T):
            nc.tensor.matmul(
                g_ps[:], Wh[:, ts(t, F)], c_col[:, t : t + 1],
                start=(t == 0), stop=(t == T - 1),
            )
        out_sb = small.tile([F, 1], FP, tag="out")
        nc.scalar.mul(out_sb[:], g_ps[:], 1.0 / N)
        nc.sync.dma_start(outd[:], out_sb[:])


def build(reps: int = 1):
    nc = bacc.Bacc(
        "TRN2", target_bir_lowering=False, debug=False,
        enable_asserts=False, num_devices=NCORES,
    )
    hb = nc.dram_tensor("hb", [N, K], FP, kind="ExternalInput").ap()
    Wd = nc.dram_tensor("W", [K, F], FP, kind="ExternalInput").ap()
    ad = nc.dram_tensor("a", [2 * F, 1], FP, kind="ExternalInput").ap()
    outd = nc.dram_tensor("out", [F, 1], FP, kind="ExternalOutput").ap()

    with tile.TileContext(nc) as tc:
        with ExitStack() as ctx:
            consts = ctx.enter_context(tc.tile_pool(name="consts", bufs=1))
            ident = consts.tile([P, P], FP)
            make_identity(nc, ident[:])
            # pull the exp ACT table load ahead of the critical path
            warm = consts.tile([P, 1], FP)
            nc.scalar.activation(warm[:], ident[:, 0:1], AF.Exp)
            W_sb = consts.tile([K, F], FP)
            nc.sync.dma_start(W_sb[:], Wd[:])
            a1 = consts.tile([F, 1], FP)
            nc.sync.dma_start(a1[:], ad[0:F, :])
            a2 = consts.tile([F, 1], FP)
            nc.sync.dma_start(a2[:], ad[F : 2 * F, :])
            for _ in range(reps):
                emit_batch(tc, outd, hb, Wd, ad, consts, ident, W_sb, a1, a2)
    nc.compile()
    return nc


_nc_cache = {}


def _get_nc(reps: int = 1):
    if reps not in _nc_cache:
        _nc_cache[reps] = build(reps)
    return _nc_cache[reps]


def kernel(h: np.ndarray, W: np.ndarray, a: np.ndarray) -> np.ndarray:
    assert h.shape == (NCORES, N, K) and W.shape == (K, F) and a.shape == (2 * F,)
    nc = _get_nc(1)
    in_maps = [
        {
            "hb": np.ascontiguousarray(h[b], dtype=np.float32),
            "W": np.ascontiguousarray(W, dtype=np.float32),
            "a": np.ascontiguousarray(a.reshape(2 * F, 1), dtype=np.float32),
        }
        for b in range(NCORES)
    ]
    res = run_bass_kernel_spmd(nc, in_maps, core_ids=list(range(NCORES)))
    out = np.stack([res.results[b]["out"].reshape(F) for b in range(NCORES)])
    return out.astype(np.float32)
